# revision 21
# baseline (speedup 1.0000x reference)
"""BitMLPBlock Trainium2 kernel: out = x + fc2(gelu(fc1(actquant(x)))).

BitNet b1.58 forward: activations per-token int8 absmax quant, weights
ternary {-1,0,1} with a global scale. Both quantized operand sets are
exactly representable in bf16 (ints <= 128), so bf16 matmuls with f32 PSUM
accumulation reproduce the f32 reference einsum exactly; the only
approximation left is the Gelu LUT and scale-division rounding.

Sharding: data-parallel over the batch dim (8 batches -> 8 NeuronCores),
weights replicated. No collectives.

Self-contained: hardcodes shapes B=8, T=8192, D=512, H=2048.
"""
import numpy as np

from concourse import bass, mybir, tile
from concourse.bass_utils import run_bass_kernel_spmd
from concourse.vector_clock import ScopedClock

B, T, D, H = 8, 8192, 512, 2048
N_CORES = 8
P = 128                      # partitions / token tile
N_TILES = T // P             # 64 token tiles per core
KD = D // P                  # 4  k-tiles for fc1
KH = H // P                  # 16 k-tiles for fc2
NC1 = H // 512               # 4  psum chunks for fc1
MAGIC = 12582912.0           # 1.5 * 2^23: float32 RNE round-to-int trick
MAGIC16 = 1536.0             # 1.5 * 2^10: fp16 RNE round-to-int trick
F32 = mybir.dt.float32
BF16 = mybir.dt.bfloat16
F16 = mybir.dt.float16


# ---------------------------------------------------------------------------
# Workarounds for this container's walrus build, which supports only ONE sync
# wait command per instruction. Tile's tail drain and its add_semaphores pass
# both emit multi-wait instructions; split the extras onto standalone
# wait/NoOp instructions on the same engine.
# ---------------------------------------------------------------------------
_PATCHED = False


def _patch_tile():
    global _PATCHED
    if _PATCHED:
        return
    _PATCHED = True

    def _drain_and_barrier_split(self, tick_clock, wait_clock):
        nc = self.nc
        probe = nc.sync.nop(nofuse=True)
        wait_clock.add_sem_waits(
            probe.ins, ScopedClock({None: tick_clock.global_clock}))
        si = probe.ins.sync_info
        waits = list(si.on_wait) if si is not None and si.on_wait else []
        sems_by_name = {}
        if self.sems is not None:
            for s in self.sems.allocated().values():
                sems_by_name[s.name] = s
        kept = []
        for w in waits:
            sem = sems_by_name.get(w.ant_name)
            if sem is None or w.wait_mode != "sem-ge-imm" or w.wait_value is None:
                kept.append(w)
                continue
            nc.sync.wait_ge(sem, w.wait_value)
        if si is not None:
            si.on_wait = kept
        nc.sync.drain()
        nc.all_engine_barrier()
        assert self.sems is not None
        popped = nc._tile_sem_poison_stack.pop()
        assert popped is self._sem_poison
        nc.clear_and_free_semaphores(list(self.sems.allocated().values()))
        nc.all_engine_barrier()

    tile.TileContext._drain_and_barrier = _drain_and_barrier_split

    orig_commit = tile.TileContext._commit_instruction

    def _commit_split_waits(self, inst, lazy_reg_writes=True):
        si = getattr(inst, "sync_info", None)
        if (
            si is not None
            and si.on_wait
            and len(si.on_wait) > 1
            and inst.engine != mybir.EngineType.Unassigned
        ):
            waits = list(si.on_wait)
            si.on_wait = [waits[-1]]
            for w in waits[:-1]:
                nop = mybir.InstNoOp(
                    name=self.nc.get_next_instruction_name(),
                    text_hint="split_wait",
                    bass_nofuse=True,
                    engine=inst.engine,
                    sync_info=mybir.SyncInfo(on_wait=[w], on_update=[]),
                )
                self._add_instruction(nop)
        return orig_commit(self, inst, lazy_reg_writes)

    tile.TileContext._commit_instruction = _commit_split_waits


_patch_tile()


def build_nc(c1: float, c2: float, with_b1: bool):
    """c1/c2: host-folded dequant consts (weight unscale / 127)."""
    nc = bass.Bass("TRN2", target_bir_lowering=False, num_devices=N_CORES)

    x_ext = nc.declare_dram_parameter("x", [T, D], F32, isOutput=False)
    w1t_ext = nc.declare_dram_parameter("w1t", [KD, P, H], F16, isOutput=False)
    w2t_ext = nc.declare_dram_parameter("w2t", [KH, P, 512], F16, isOutput=False)
    csum2_ext = nc.declare_dram_parameter("csum2", [1, 512], F16, isOutput=False)
    b1_ext = None
    if with_b1:
        b1_ext = nc.declare_dram_parameter("b1bc", [P, H], F32, isOutput=False)
    out_ext = nc.declare_dram_parameter("out", [T, D], F32, isOutput=True)

    mm = nc.tensor.matmul
    Alu = mybir.AluOpType
    Act = mybir.ActivationFunctionType

    with tile.TileContext(nc) as tc:
        with (
            tc.tile_pool(name="const", bufs=1) as cpool,
            tc.tile_pool(name="xin", bufs=8) as xpool,
            tc.tile_pool(name="vec", bufs=12) as vpool,
            tc.tile_pool(name="stage", bufs=6) as spool,
            tc.tile_pool(name="big", bufs=4) as bpool,
            tc.tile_pool(name="outp", bufs=4) as opool,
            tc.tile_pool(name="ps_mm1", bufs=4, space="PSUM") as ps_mm1,
            tc.tile_pool(name="ps_2", bufs=4, space="PSUM") as ps_2,
        ):
            # resident weights + fc2 offset-correction operands
            w1t_sb = cpool.tile([P, KD, H], F16, tag="w1")
            w2t_sb = cpool.tile([P, KH, 512], F16, tag="w2")
            for j in range(KD):
                nc.gpsimd.dma_start(out=w1t_sb[:, j, :], in_=w1t_ext[j])
            for k in range(KH):
                nc.gpsimd.dma_start(out=w2t_sb[:, k, :], in_=w2t_ext[k])
            csum2_sb = cpool.tile([1, 512], F16, tag="csum2")
            nc.gpsimd.dma_start(out=csum2_sb[:, :], in_=csum2_ext[:, :])
            ones_mag = cpool.tile([1, P], F16, tag="ones")
            nc.vector.memset(ones_mag[:, :], MAGIC16)
            b1_sb = None
            if with_b1:
                b1_sb = cpool.tile([P, H], F32, tag="b1")
                nc.gpsimd.dma_start(out=b1_sb[:, :], in_=b1_ext[:, :])

            def stage_a(t):
                """Load + act-quant + transpose of x for tile t."""
                row = t * P
                x_t = xpool.tile([P, D], F32, tag="x")
                nc.gpsimd.dma_start(out=x_t[:, :], in_=x_ext[row:row + P, :])

                amax = vpool.tile([P, 1], F32, tag="amax")
                nc.vector.tensor_reduce(
                    amax[:, :], x_t[:, :], axis=mybir.AxisListType.X,
                    op=Alu.max, apply_absolute_value=True)
                t1 = vpool.tile([P, 1], F32, tag="t1")
                nc.vector.tensor_scalar(
                    t1[:, :], amax[:, :], 1e-5, 1.0 / 127.0,
                    op0=Alu.max, op1=Alu.mult)
                inv1 = vpool.tile([P, 1], F32, tag="inv1")
                nc.vector.tensor_scalar_mul(inv1[:, :], t1[:, :], c1 * 127.0)
                s_x = vpool.tile([P, 1], F32, tag="sx")
                nc.vector.reciprocal(s_x[:, :], t1[:, :])

                xr = spool.tile([P, D], F16, tag="xr")
                nc.scalar.activation(
                    xr[:, :], x_t[:, :], Act.Copy, bias=MAGIC16, scale=s_x[:, :])
                xq = spool.tile([P, D], F16, tag="xq")
                nc.vector.tensor_scalar(
                    xq[:, :], xr[:, :], MAGIC16, None, op0=Alu.subtract)

                xT = spool.tile([P, KD, P], F16, tag="xT")
                nc.sync.dma_start_transpose(out=xT[:, :, :], in_=xq[:, :])
                return x_t, inv1, xT

            def stage_b(t, x_t, inv1, xT):
                """fc1 -> gelu -> h-quant -> hT transpose for tile t."""
                h_sb = bpool.tile([P, H], F32, tag="h")
                amax4 = vpool.tile([P, NC1], F32, tag="amax4")
                for c in range(NC1):
                    ps1 = ps_mm1.tile([P, 512], F32, tag="mm1")
                    for j in range(KD):
                        mm(ps1[:, :], xT[:, j, :], w1t_sb[:, j, c * 512:(c + 1) * 512],
                           start=(j == 0), stop=(j == KD - 1))
                    if with_b1:
                        hlin = bpool.tile([P, 512], F32, tag="hlin")
                        nc.scalar.activation(
                            hlin[:, :], ps1[:, :], Act.Copy, bias=0.0,
                            scale=inv1[:, :])
                        hb = bpool.tile([P, 512], F32, tag="hb")
                        nc.vector.tensor_add(
                            hb[:, :], hlin[:, :], b1_sb[:, c * 512:(c + 1) * 512])
                        nc.scalar.activation(
                            h_sb[:, c * 512:(c + 1) * 512], hb[:, :], Act.Gelu,
                            bias=0.0, scale=1.0)
                    else:
                        nc.scalar.activation(
                            h_sb[:, c * 512:(c + 1) * 512], ps1[:, :], Act.Gelu,
                            bias=0.0, scale=inv1[:, :])
                    nc.vector.tensor_reduce(
                        amax4[:, c:c + 1], h_sb[:, c * 512:(c + 1) * 512],
                        axis=mybir.AxisListType.X,
                        op=Alu.max, apply_absolute_value=True)

                # ---- act quant of h ----
                amax_h = vpool.tile([P, 1], F32, tag="amaxh")
                nc.vector.tensor_reduce(
                    amax_h[:, :], amax4[:, :], axis=mybir.AxisListType.X,
                    op=Alu.max, apply_absolute_value=True)
                t2 = vpool.tile([P, 1], F32, tag="t2")
                nc.vector.tensor_scalar(
                    t2[:, :], amax_h[:, :], 1e-5, 1.0 / 127.0,
                    op0=Alu.max, op1=Alu.mult)
                inv2 = vpool.tile([P, 1], F32, tag="inv2")
                nc.vector.tensor_scalar_mul(inv2[:, :], t2[:, :], c2 * 127.0)
                s_h = vpool.tile([P, 1], F32, tag="sh")
                nc.vector.reciprocal(s_h[:, :], t2[:, :])

                # single-op round: fp16 output snaps (h*s_h + 1536) to the
                # integer grid; the +1536 offset is removed inside fc2 via a
                # K=1 corrective matmul against -1536*colsum(w2).
                # Split into quarters so fc2 starts while later quarters still
                # quantize/transpose.
                NQ = 2
                HQ = H // NQ
                KQ = KH // NQ
                hT_parts = []
                for q in range(NQ):
                    hq_q = bpool.tile([P, HQ], F16, tag=f"hq{q}")
                    nc.scalar.activation(
                        hq_q[:, :], h_sb[:, q * HQ:(q + 1) * HQ],
                        Act.Copy, bias=MAGIC16, scale=s_h[:, :])
                    hT_q = bpool.tile([P, KQ, P], F16, tag=f"hT{q}")
                    nc.sync.dma_start_transpose(out=hT_q[:, :, :], in_=hq_q[:, :])
                    hT_parts.append(hT_q)
                return hT_parts, inv2

            def stage_c(t, x_t, hT_parts, inv2):
                """fc2 (+offset correction) -> dequant -> residual -> store."""
                row = t * P
                KQ = KH // len(hT_parts)
                ps2 = ps_2.tile([P, 512], F32, tag="mm2")
                for k in range(KH):
                    hT_q = hT_parts[k // KQ]
                    mm(ps2[:, :], hT_q[:, k % KQ, :], w2t_sb[:, k, :],
                       start=(k == 0), stop=False)
                mm(ps2[:, :], ones_mag[:, :], csum2_sb[:, :],
                   start=False, stop=True)
                out_t = opool.tile([P, D], F32, tag="out")
                nc.vector.scalar_tensor_tensor(
                    out_t[:, :], ps2[:, :], inv2[:, :], x_t[:, :],
                    op0=Alu.mult, op1=Alu.add)
                nc.gpsimd.dma_start(out=out_ext[row:row + P, :], in_=out_t[:, :])

            # 3-stage software pipeline. Creation (= priority) order interleaves
            # fc1 of tiles t+1, t+2 between fc2(t-1) and fc2(t) on the PE, so
            # the hq-gated hT transpose of tile t has a full fc1's worth of PE
            # work hiding its latency. Stage A runs further ahead so the sync
            # queue always has future xT transposes issued before it blocks on
            # an hT wait.
            LA_A = 4   # stage_a lookahead vs stage_b
            LA_B = 2   # stage_b lookahead vs stage_c
            a_out = {}
            b_out = {}
            for t in range(N_TILES + LA_A + LA_B):
                ta = t
                tb = t - LA_A
                tc = t - LA_A - LA_B
                if ta < N_TILES:
                    a_out[ta] = stage_a(ta)
                if 0 <= tb < N_TILES:
                    x_t, inv1, xT = a_out[tb]
                    b_out[tb] = stage_b(tb, x_t, inv1, xT)
                if 0 <= tc < N_TILES:
                    x_t, inv1, xT = a_out.pop(tc)
                    hT_parts, inv2 = b_out.pop(tc)
                    stage_c(tc, x_t, hT_parts, inv2)

    return nc


def _host_weight_quant(w):
    w = np.asarray(w, np.float32)
    scale = 1.0 / np.float32(max(np.mean(np.abs(w), dtype=np.float32), 1e-5))
    tern = np.clip(np.round(w * scale), -1.0, 1.0).astype(np.float32)
    unscale = np.float32(1.0) / scale  # matches reference's division by scale
    return tern, float(unscale)


LAST_RESULTS = None  # test-harness hook: BassKernelResults of last kernel() run


def kernel(x, w1, b1, w2, b2, _trace=False):
    global LAST_RESULTS
    import ml_dtypes
    bf16 = ml_dtypes.bfloat16

    x = np.asarray(x, np.float32)
    w1_tern, w1_un = _host_weight_quant(w1)
    w2_tern, w2_un = _host_weight_quant(w2)
    c1 = float(np.float32(w1_un) / np.float32(127.0))
    c2 = float(np.float32(w2_un) / np.float32(127.0))

    w1t = np.ascontiguousarray(w1_tern.T).reshape(KD, P, H).astype(np.float16)
    w2t = np.ascontiguousarray(w2_tern.T).reshape(KH, P, 512).astype(np.float16)
    csum2 = (-w2_tern.sum(axis=1, dtype=np.float64)).astype(np.float16)
    csum2 = csum2.reshape(1, 512)

    b1 = np.asarray(b1, np.float32)
    b2 = np.asarray(b2, np.float32)
    with_b1 = bool(np.any(b1))

    nc = build_nc(c1, c2, with_b1)

    in_maps = []
    for core in range(N_CORES):
        m = {
            "x": np.ascontiguousarray(x[core]),
            "w1t": w1t,
            "w2t": w2t,
            "csum2": csum2,
        }
        if with_b1:
            m["b1bc"] = np.ascontiguousarray(
                np.broadcast_to(b1, (P, H)).astype(np.float32))
        in_maps.append(m)

    res = run_bass_kernel_spmd(
        nc, in_maps, core_ids=list(range(N_CORES)), trace=_trace)
    LAST_RESULTS = res
    out = np.stack([res.results[c]["out"] for c in range(N_CORES)], axis=0)
    if np.any(b2):
        out = out + b2[None, None, :]
    return out.astype(np.float32)


# revision 24
# speedup vs baseline: 1.3520x; 1.3520x over previous
"""BitMLPBlock Trainium2 kernel: out = x + fc2(gelu(fc1(actquant(x)))).

BitNet b1.58 forward: activations per-token int8 absmax quant, weights
ternary {-1,0,1} with a global scale. Both quantized operand sets are
exactly representable in bf16 (ints <= 128), so bf16 matmuls with f32 PSUM
accumulation reproduce the f32 reference einsum exactly; the only
approximation left is the Gelu LUT and scale-division rounding.

Sharding: data-parallel over the batch dim (8 batches -> 8 NeuronCores),
weights replicated. No collectives.

Self-contained: hardcodes shapes B=8, T=8192, D=512, H=2048.
"""
import numpy as np

from concourse import bass, mybir, tile
from concourse.bass_utils import run_bass_kernel_spmd
from concourse.vector_clock import ScopedClock

B, T, D, H = 8, 8192, 512, 2048
N_CORES = 8
P = 128                      # partitions / token tile
N_TILES = T // P             # 64 token tiles per core
KD = D // P                  # 4  k-tiles for fc1
KH = H // P                  # 16 k-tiles for fc2
NC1 = H // 512               # 4  psum chunks for fc1
MAGIC = 12582912.0           # 1.5 * 2^23: float32 RNE round-to-int trick
MAGIC16 = 1536.0             # 1.5 * 2^10: fp16 RNE round-to-int trick
F32 = mybir.dt.float32
BF16 = mybir.dt.bfloat16
F16 = mybir.dt.float16


# ---------------------------------------------------------------------------
# Workarounds for this container's walrus build, which supports only ONE sync
# wait command per instruction. Tile's tail drain and its add_semaphores pass
# both emit multi-wait instructions; split the extras onto standalone
# wait/NoOp instructions on the same engine.
# ---------------------------------------------------------------------------
_PATCHED = False


def _patch_tile():
    global _PATCHED
    if _PATCHED:
        return
    _PATCHED = True

    def _drain_and_barrier_split(self, tick_clock, wait_clock):
        nc = self.nc
        probe = nc.sync.nop(nofuse=True)
        wait_clock.add_sem_waits(
            probe.ins, ScopedClock({None: tick_clock.global_clock}))
        si = probe.ins.sync_info
        waits = list(si.on_wait) if si is not None and si.on_wait else []
        sems_by_name = {}
        if self.sems is not None:
            for s in self.sems.allocated().values():
                sems_by_name[s.name] = s
        kept = []
        for w in waits:
            sem = sems_by_name.get(w.ant_name)
            if sem is None or w.wait_mode != "sem-ge-imm" or w.wait_value is None:
                kept.append(w)
                continue
            nc.sync.wait_ge(sem, w.wait_value)
        if si is not None:
            si.on_wait = kept
        nc.sync.drain()
        nc.all_engine_barrier()
        assert self.sems is not None
        popped = nc._tile_sem_poison_stack.pop()
        assert popped is self._sem_poison
        nc.clear_and_free_semaphores(list(self.sems.allocated().values()))
        nc.all_engine_barrier()

    tile.TileContext._drain_and_barrier = _drain_and_barrier_split

    orig_commit = tile.TileContext._commit_instruction

    def _commit_split_waits(self, inst, lazy_reg_writes=True):
        si = getattr(inst, "sync_info", None)
        if (
            si is not None
            and si.on_wait
            and len(si.on_wait) > 1
            and inst.engine != mybir.EngineType.Unassigned
        ):
            waits = list(si.on_wait)
            si.on_wait = [waits[-1]]
            for w in waits[:-1]:
                nop = mybir.InstNoOp(
                    name=self.nc.get_next_instruction_name(),
                    text_hint="split_wait",
                    bass_nofuse=True,
                    engine=inst.engine,
                    sync_info=mybir.SyncInfo(on_wait=[w], on_update=[]),
                )
                self._add_instruction(nop)
        return orig_commit(self, inst, lazy_reg_writes)

    tile.TileContext._commit_instruction = _commit_split_waits


_patch_tile()


def build_nc(c1: float, c2: float, with_b1: bool):
    """c1/c2: host-folded dequant consts (weight unscale / 127)."""
    nc = bass.Bass("TRN2", target_bir_lowering=False, num_devices=N_CORES)

    x_ext = nc.declare_dram_parameter("x", [T, D], F32, isOutput=False)
    w1t_ext = nc.declare_dram_parameter("w1t", [KD, P, H], F16, isOutput=False)
    w2t_ext = nc.declare_dram_parameter("w2t", [KH, P, 512], F16, isOutput=False)
    csum2_ext = nc.declare_dram_parameter("csum2", [1, 512], F16, isOutput=False)
    b1_ext = None
    if with_b1:
        b1_ext = nc.declare_dram_parameter("b1bc", [P, H], F32, isOutput=False)
    out_ext = nc.declare_dram_parameter("out", [T, D], F32, isOutput=True)

    mm = nc.tensor.matmul
    Alu = mybir.AluOpType
    Act = mybir.ActivationFunctionType

    with tile.TileContext(nc) as tc:
        with (
            tc.tile_pool(name="const", bufs=1) as cpool,
            tc.tile_pool(name="xin", bufs=8) as xpool,
            tc.tile_pool(name="vec", bufs=12) as vpool,
            tc.tile_pool(name="stage", bufs=6) as spool,
            tc.tile_pool(name="big", bufs=4) as bpool,
            tc.tile_pool(name="outp", bufs=4) as opool,
            tc.tile_pool(name="ps_mm1", bufs=6, space="PSUM") as ps_mm1,
            tc.tile_pool(name="ps_2", bufs=2, space="PSUM") as ps_2,
        ):
            # resident weights + fc2 offset-correction operands
            w1t_sb = cpool.tile([P, KD, H], F16, tag="w1")
            w2t_sb = cpool.tile([P, KH, 512], F16, tag="w2")
            for j in range(KD):
                nc.gpsimd.dma_start(out=w1t_sb[:, j, :], in_=w1t_ext[j])
            for k in range(KH):
                nc.gpsimd.dma_start(out=w2t_sb[:, k, :], in_=w2t_ext[k])
            csum2_sb = cpool.tile([1, 512], F16, tag="csum2")
            nc.gpsimd.dma_start(out=csum2_sb[:, :], in_=csum2_ext[:, :])
            ones_mag = cpool.tile([1, P], F16, tag="ones")
            nc.vector.memset(ones_mag[:, :], MAGIC16)
            b1_sb = None
            if with_b1:
                b1_sb = cpool.tile([P, H], F32, tag="b1")
                nc.gpsimd.dma_start(out=b1_sb[:, :], in_=b1_ext[:, :])

            def stage_a(t):
                """Load + act-quant + transpose of x for tile t."""
                row = t * P
                x_t = xpool.tile([P, D], F32, tag="x")
                nc.gpsimd.dma_start(out=x_t[:, :], in_=x_ext[row:row + P, :])

                amax = vpool.tile([P, 1], F32, tag="amax")
                nc.vector.tensor_reduce(
                    amax[:, :], x_t[:, :], axis=mybir.AxisListType.X,
                    op=Alu.max, apply_absolute_value=True)
                t1 = vpool.tile([P, 1], F32, tag="t1")
                nc.vector.tensor_scalar(
                    t1[:, :], amax[:, :], 1e-5, 1.0 / 127.0,
                    op0=Alu.max, op1=Alu.mult)
                inv1 = vpool.tile([P, 1], F32, tag="inv1")
                nc.vector.tensor_scalar_mul(inv1[:, :], t1[:, :], c1 * 127.0)
                s_x = vpool.tile([P, 1], F32, tag="sx")
                nc.vector.reciprocal(s_x[:, :], t1[:, :])

                xr = spool.tile([P, D], F16, tag="xr")
                nc.scalar.activation(
                    xr[:, :], x_t[:, :], Act.Copy, bias=MAGIC16, scale=s_x[:, :])
                xq = spool.tile([P, D], F16, tag="xq")
                nc.vector.tensor_scalar(
                    xq[:, :], xr[:, :], MAGIC16, None, op0=Alu.subtract)

                xT = spool.tile([P, KD, P], F16, tag="xT")
                nc.sync.dma_start_transpose(out=xT[:, :, :], in_=xq[:, :])
                return x_t, inv1, xT

            def stage_b(t, x_t, inv1, xT):
                """fc1 -> gelu -> h-quant -> fc2 -> residual -> store."""
                row = t * P
                h_sb = bpool.tile([P, H], F32, tag="h")
                amax4 = vpool.tile([P, NC1], F32, tag="amax4")
                ps1_banks = []
                for _c in range(NC1):
                    ps1_c = ps_mm1.tile([P, 512], F32, tag="mm1")
                    ps1_banks.append(ps1_c)
                for j in range(KD):
                    for c in range(NC1):
                        mm(ps1_banks[c][:, :], xT[:, j, :],
                           w1t_sb[:, j, c * 512:(c + 1) * 512],
                           start=(j == 0), stop=(j == KD - 1))
                for c in range(NC1):
                    ps1 = ps1_banks[c]
                    if with_b1:
                        hlin = bpool.tile([P, 512], F32, tag="hlin")
                        nc.scalar.activation(
                            hlin[:, :], ps1[:, :], Act.Copy, bias=0.0,
                            scale=inv1[:, :])
                        hb = bpool.tile([P, 512], F32, tag="hb")
                        nc.vector.tensor_add(
                            hb[:, :], hlin[:, :], b1_sb[:, c * 512:(c + 1) * 512])
                        nc.scalar.activation(
                            h_sb[:, c * 512:(c + 1) * 512], hb[:, :], Act.Gelu,
                            bias=0.0, scale=1.0)
                    else:
                        nc.scalar.activation(
                            h_sb[:, c * 512:(c + 1) * 512], ps1[:, :], Act.Gelu,
                            bias=0.0, scale=inv1[:, :])
                    nc.vector.tensor_reduce(
                        amax4[:, c:c + 1], h_sb[:, c * 512:(c + 1) * 512],
                        axis=mybir.AxisListType.X,
                        op=Alu.max, apply_absolute_value=True)

                # ---- act quant of h ----
                amax_h = vpool.tile([P, 1], F32, tag="amaxh")
                nc.vector.tensor_reduce(
                    amax_h[:, :], amax4[:, :], axis=mybir.AxisListType.X,
                    op=Alu.max, apply_absolute_value=True)
                t2 = vpool.tile([P, 1], F32, tag="t2")
                nc.vector.tensor_scalar(
                    t2[:, :], amax_h[:, :], 1e-5, 1.0 / 127.0,
                    op0=Alu.max, op1=Alu.mult)
                inv2 = vpool.tile([P, 1], F32, tag="inv2")
                nc.vector.tensor_scalar_mul(inv2[:, :], t2[:, :], c2 * 127.0)
                s_h = vpool.tile([P, 1], F32, tag="sh")
                nc.vector.reciprocal(s_h[:, :], t2[:, :])

                # single-op round: fp16 output snaps (h*s_h + 1536) to the
                # integer grid; the +1536 offset is removed inside fc2 via a
                # K=1 corrective matmul against -1536*colsum(w2).
                # Split into quarters so fc2 starts while later quarters still
                # quantize/transpose.
                NQ = 2
                HQ = H // NQ
                KQ = KH // NQ
                hT_parts = []
                for q in range(NQ):
                    hq_q = bpool.tile([P, HQ], F16, tag=f"hq{q}")
                    nc.scalar.activation(
                        hq_q[:, :], h_sb[:, q * HQ:(q + 1) * HQ],
                        Act.Copy, bias=MAGIC16, scale=s_h[:, :])
                    hT_q = bpool.tile([P, KQ, P], F16, tag=f"hT{q}")
                    nc.sync.dma_start_transpose(out=hT_q[:, :, :], in_=hq_q[:, :])
                    hT_parts.append(hT_q)

                # ---- fc2 (+offset correction) + dequant + residual ----
                ps2 = ps_2.tile([P, 512], F32, tag="mm2")
                for k in range(KH):
                    hT_q = hT_parts[k // KQ]
                    mm(ps2[:, :], hT_q[:, k % KQ, :], w2t_sb[:, k, :],
                       start=(k == 0), stop=False)
                mm(ps2[:, :], ones_mag[:, :], csum2_sb[:, :],
                   start=False, stop=True)
                out_t = opool.tile([P, D], F32, tag="out")
                nc.vector.scalar_tensor_tensor(
                    out_t[:, :], ps2[:, :], inv2[:, :], x_t[:, :],
                    op0=Alu.mult, op1=Alu.add)
                nc.gpsimd.dma_start(out=out_ext[row:row + P, :], in_=out_t[:, :])

            # software pipeline: stage A runs LOOKAHEAD tiles ahead so the
            # sync queue has future xT transposes in flight before it blocks
            # on the current tile's hq-gated hT transpose.
            LOOKAHEAD = 2
            pending = []
            for t in range(min(LOOKAHEAD, N_TILES)):
                pending.append((t, *stage_a(t)))
            for t in range(N_TILES):
                if t + LOOKAHEAD < N_TILES:
                    pending.append((t + LOOKAHEAD, *stage_a(t + LOOKAHEAD)))
                stage_b(*pending.pop(0))

    return nc


def _host_weight_quant(w):
    w = np.asarray(w, np.float32)
    scale = 1.0 / np.float32(max(np.mean(np.abs(w), dtype=np.float32), 1e-5))
    tern = np.clip(np.round(w * scale), -1.0, 1.0).astype(np.float32)
    unscale = np.float32(1.0) / scale  # matches reference's division by scale
    return tern, float(unscale)


LAST_RESULTS = None  # test-harness hook: BassKernelResults of last kernel() run


def kernel(x, w1, b1, w2, b2, _trace=False):
    global LAST_RESULTS
    import ml_dtypes
    bf16 = ml_dtypes.bfloat16

    x = np.asarray(x, np.float32)
    w1_tern, w1_un = _host_weight_quant(w1)
    w2_tern, w2_un = _host_weight_quant(w2)
    c1 = float(np.float32(w1_un) / np.float32(127.0))
    c2 = float(np.float32(w2_un) / np.float32(127.0))

    w1t = np.ascontiguousarray(w1_tern.T).reshape(KD, P, H).astype(np.float16)
    w2t = np.ascontiguousarray(w2_tern.T).reshape(KH, P, 512).astype(np.float16)
    csum2 = (-w2_tern.sum(axis=1, dtype=np.float64)).astype(np.float16)
    csum2 = csum2.reshape(1, 512)

    b1 = np.asarray(b1, np.float32)
    b2 = np.asarray(b2, np.float32)
    with_b1 = bool(np.any(b1))

    nc = build_nc(c1, c2, with_b1)

    in_maps = []
    for core in range(N_CORES):
        m = {
            "x": np.ascontiguousarray(x[core]),
            "w1t": w1t,
            "w2t": w2t,
            "csum2": csum2,
        }
        if with_b1:
            m["b1bc"] = np.ascontiguousarray(
                np.broadcast_to(b1, (P, H)).astype(np.float32))
        in_maps.append(m)

    res = run_bass_kernel_spmd(
        nc, in_maps, core_ids=list(range(N_CORES)), trace=_trace)
    LAST_RESULTS = res
    out = np.stack([res.results[c]["out"] for c in range(N_CORES)], axis=0)
    if np.any(b2):
        out = out + b2[None, None, :]
    return out.astype(np.float32)
